# revision 14
# baseline (speedup 1.0000x reference)
"""FLaGPE node encoder on 8 Trainium2 NeuronCores.

Sharding: data parallel over the graph axis, 2 graphs per core; the
small MLP/LayerNorm/linear parameters are replicated.

Algorithm: the reference builds dense random-walk stacks
rw = [I, P, ..., P^15] ([K,G,N,N]) but only consumes
(rw * blend).mean(-1), where blend = a + (1-2a)*[frag_i == frag_j].
With F = onehot(frag) ([N,32]) this collapses to

    feat[k,i] = (1/N) * ( a * (P^k 1)[i] + (1-2a) * (P^k F)[i, frag_i] )

so only M_k = P^k @ [F, 1] ([N,33]) is needed: 15 thin matmuls per
graph instead of dense N x N matrix powers.

Adjacency (duplicate edges counted) is built on-device as
adjT = V^T U from fp16 one-hot edge encodings on the tensor engine
(PSUM accumulates exact integer counts; fp16 holds them exactly).
deg falls out of the first iteration's "ones" column; the row
normalization 1/max(deg,1) rides the PSUM->SBUF copy (per-partition
scalar multiply on the vector engine).

Schedule: edge DMAs + both graphs' adjacency builds run first
(tensor engine back-to-back on 512-wide fp16 matmuls), then the two
graphs' 15 power-iteration steps are interleaved so each graph's
serial chain hides in the other's gaps; hx = x@Wx+bx fills leftover
tensor-engine slack.  Extraction is batched: M_k for 4 consecutive k
lands in one [128,4,33] buffer, one multiply (weights broadcast via
stride-0 AP) + one reduce per block of 4 steps.
"""

import numpy as np

import concourse.bacc as bacc
import concourse.bass as bass
import concourse.tile as tile
from concourse import mybir
from concourse.masks import make_identity
from concourse.bass_utils import run_bass_kernel_spmd

FP32, FP16, I32 = mybir.dt.float32, mybir.dt.float16, mybir.dt.int32
FP32R = mybir.dt.float32r
AF = mybir.ActivationFunctionType
OP = mybir.AluOpType

P = 128
G, N, E, K = 16, 512, 4096, 16
NF = 32                     # fragment classes
DIN, DPE, HID = 64, 28, 64
DX = 100                    # dim_emb - dim_pe
DOUT = DX + DPE             # 128
NCORES = 8
GPC = G // NCORES           # graphs per core = 2
NB = N // P                 # 4 node blocks / graph
EC = E // P                 # 32 edge chunks / graph
XB = GPC * N // P           # 8 x blocks / core
LN_EPS = 1e-5
MC = NF + 1                 # M columns: 32 one-hot + 1 ones
KB = 4                      # extraction batch (k's per M buffer)


def _bc4(ap, n):
    """[P, m] AP -> [P, n, m] with stride-0 middle dim."""
    return bass.AP(tensor=ap.tensor, offset=ap.offset,
                   ap=[ap.ap[0], [0, n], ap.ap[1]])


def _build():
    nc = bacc.Bacc()
    x_d = nc.declare_dram_parameter("x", [GPC * N, DIN], FP32, isOutput=False)
    e_d = nc.declare_dram_parameter("edges", [GPC, 2, E], I32, isOutput=False)
    f_d = nc.declare_dram_parameter("frags", [GPC, N], I32, isOutput=False)
    al_d = nc.declare_dram_parameter("alpha", [1, 1], FP32, isOutput=False)
    wx_d = nc.declare_dram_parameter("Wx", [DIN, DX], FP32, isOutput=False)
    bx_d = nc.declare_dram_parameter("bx", [1, DX], FP32, isOutput=False)
    w1_d = nc.declare_dram_parameter("W1", [K, HID], FP32, isOutput=False)
    b1_d = nc.declare_dram_parameter("b1", [HID, 1], FP32, isOutput=False)
    w2_d = nc.declare_dram_parameter("W2", [HID, HID], FP32, isOutput=False)
    b2_d = nc.declare_dram_parameter("b2", [HID, 1], FP32, isOutput=False)
    w3_d = nc.declare_dram_parameter("W3", [HID, DPE], FP32, isOutput=False)
    b3_d = nc.declare_dram_parameter("b3", [DPE, 1], FP32, isOutput=False)
    ga_d = nc.declare_dram_parameter("gamma", [1, DPE], FP32, isOutput=False)
    be_d = nc.declare_dram_parameter("beta", [1, DPE], FP32, isOutput=False)
    out_d = nc.declare_dram_parameter("out", [GPC * N, DOUT], FP32, isOutput=True)

    def bcast(h, n):
        a = h[0:1, 0:n]
        return bass.AP(tensor=a.tensor, offset=a.offset, ap=[[0, P], [1, n]])

    with tile.TileContext(nc) as tc:
        with (
            tc.tile_pool(name="consts", bufs=1) as consts,
            tc.tile_pool(name="epool", bufs=2) as epool,
            tc.tile_pool(name="ohp", bufs=6) as ohp,
            tc.tile_pool(name="adjp", bufs=2 * NB) as adjp,
            tc.tile_pool(name="mpool", bufs=2 * NB * GPC) as mpool,
            tc.tile_pool(name="fpool", bufs=2 * NB) as fpool,
            tc.tile_pool(name="spool", bufs=8) as spool,
            tc.tile_pool(name="opool", bufs=XB) as opool,
            tc.tile_pool(name="ps8", bufs=8, space="PSUM") as ps8,
        ):
            def pst(shape, name):
                return ps8.tile(shape, FP32, tag="ps", name=name)

            # ---------------- edge DMAs first ----------------
            ED = {}
            for g in range(GPC):
                src_i = epool.tile([P, EC], I32, tag="srci", name=f"srci{g}")
                nc.sync.dma_start(
                    out=src_i, in_=e_d[g, 0].rearrange("(j c) -> j c", c=EC))
                dst_i = epool.tile([P, EC], I32, tag="dsti", name=f"dsti{g}")
                nc.sync.dma_start(
                    out=dst_i, in_=e_d[g, 1].rearrange("(j c) -> j c", c=EC))
                fr_i = epool.tile([P, NB], I32, tag="fri", name=f"fri{g}")
                nc.sync.dma_start(
                    out=fr_i, in_=f_d[g].rearrange("(b p) -> p b", p=P))
                ED[g] = (src_i, dst_i, fr_i)

            # ---------------- constants ----------------
            al_sb = consts.tile([1, 1], FP32)
            nc.sync.dma_start(out=al_sb, in_=al_d[:, :])
            iota_i = consts.tile([P, N], I32)
            nc.gpsimd.iota(iota_i, pattern=[[1, N]], base=0, channel_multiplier=0)
            ident = consts.tile([P, P], FP32)
            make_identity(nc, ident)
            iota16 = consts.tile([P, N], FP16)
            nc.vector.tensor_copy(iota16, iota_i)
            ones_row = consts.tile([1, P], FP32)
            nc.vector.memset(ones_row, 1.0)
            eps_sb = consts.tile([P, 1], FP32)
            nc.vector.memset(eps_sb, LN_EPS)

            w1_sb = consts.tile([K, HID], FP32)
            nc.scalar.dma_start(out=w1_sb, in_=w1_d[:, :])
            w2_sb = consts.tile([HID, HID], FP32)
            nc.scalar.dma_start(out=w2_sb, in_=w2_d[:, :])
            w3_sb = consts.tile([HID, DPE], FP32)
            nc.scalar.dma_start(out=w3_sb, in_=w3_d[:, :])
            b1_sb = consts.tile([HID, 1], FP32)
            nc.scalar.dma_start(out=b1_sb, in_=b1_d[:, :])
            b2_sb = consts.tile([HID, 1], FP32)
            nc.scalar.dma_start(out=b2_sb, in_=b2_d[:, :])
            b3_sb = consts.tile([DPE, 1], FP32)
            nc.scalar.dma_start(out=b3_sb, in_=b3_d[:, :])
            ga_sb = consts.tile([P, DPE], FP32)
            nc.scalar.dma_start(out=ga_sb, in_=bcast(ga_d, DPE))
            be_sb = consts.tile([P, DPE], FP32)
            nc.scalar.dma_start(out=be_sb, in_=bcast(be_d, DPE))
            w1_16 = consts.tile([K, HID], FP16)
            nc.vector.tensor_copy(w1_16, w1_sb)
            w2_16 = consts.tile([HID, HID], FP16)
            nc.vector.tensor_copy(w2_16, w2_sb)
            w3_16 = consts.tile([HID, DPE], FP16)
            nc.vector.tensor_copy(w3_16, w3_sb)
            wxb_sb = consts.tile([DIN + 1, DX], FP32)
            nc.scalar.dma_start(out=wxb_sb[0:DIN, :], in_=wx_d[:, :])
            nc.scalar.dma_start(out=wxb_sb[DIN:DIN + 1, :], in_=bx_d[:, :])

            a_sb = consts.tile([1, 1], FP32)
            nc.scalar.activation(out=a_sb, in_=al_sb, func=AF.Sigmoid)

            # ---------------- emitters ----------------
            ST = {g: {} for g in range(GPC)}
            ots = [opool.tile([P, DOUT], FP32, tag="ot", name=f"ot{i}")
                   for i in range(XB)]
            xT_sb = consts.tile([DIN + 1, GPC * N], FP32)
            nc.vector.memset(xT_sb[DIN:DIN + 1, :], 1.0)

            def emit_prep(g):
                st = ST[g]
                src_i, dst_i, fr_i = ED[g]
                src_f = epool.tile([P, EC], FP32, tag="srcf", name=f"srcf{g}")
                nc.vector.tensor_copy(src_f, src_i)
                dst_f = epool.tile([P, EC], FP32, tag="dstf", name=f"dstf{g}")
                nc.vector.tensor_copy(dst_f, dst_i)
                fr_f = epool.tile([P, NB], FP32, tag="frf", name=f"frf{g}")
                nc.vector.tensor_copy(fr_f, fr_i)
                st["src_f"], st["dst_f"] = src_f, dst_f
                F16, Mb0, Feat = [], [], []
                for b in range(NB):
                    f16 = fpool.tile([P, NF], FP16, tag="f16",
                                     name=f"f16_{g}_{b}")
                    nc.vector.tensor_scalar(
                        out=f16, in0=iota16[:, :NF], scalar1=fr_f[:, b:b + 1],
                        scalar2=None, op0=OP.is_equal)
                    mb = mpool.tile([P, KB, MC], FP16, tag="m4",
                                    name=f"mb{g}_{b}_0")
                    nc.vector.tensor_copy(mb[:, 0, :NF], f16)
                    nc.vector.memset(mb[:, 0, NF:MC], 1.0)
                    ft = fpool.tile([P, K], FP32, tag="feat",
                                    name=f"ft{g}_{b}")
                    F16.append(f16)
                    Mb0.append(mb)
                    Feat.append(ft)
                st["F16"], st["Feat"] = F16, Feat
                st["mb"] = Mb0
                st["mprev"] = list(Mb0)

            def emit_adj_start(g):
                ST[g]["psa"] = [pst([P, N], f"psa{g}_{j}") for j in range(NB)]

            def emit_adj_chunk(g, c):
                st = ST[g]
                u16 = ohp.tile([P, N], FP16, tag="u16")
                nc.vector.tensor_scalar(
                    out=u16, in0=iota16, scalar1=st["src_f"][:, c:c + 1],
                    scalar2=None, op0=OP.is_equal)
                v16 = ohp.tile([P, N], FP16, tag="v16")
                nc.vector.tensor_scalar(
                    out=v16, in0=iota16, scalar1=st["dst_f"][:, c:c + 1],
                    scalar2=None, op0=OP.is_equal)
                for jb in range(NB):
                    nc.tensor.matmul(
                        st["psa"][jb], v16[:, jb * P:(jb + 1) * P], u16,
                        start=(c == 0), stop=(c == EC - 1))

            def emit_adj_copy(g):
                adjT = []
                for jb in range(NB):
                    at = adjp.tile([P, N], FP16, tag="adjT")
                    nc.scalar.copy(at, ST[g]["psa"][jb])
                    adjT.append(at)
                ST[g]["adjT"] = adjT

            def emit_c12():
                # c1 = (1-2a)/N, c2 = a/N; broadcast across partitions
                # via PE outer product with ones.
                c12 = consts.tile([1, 2], FP32)
                nc.vector.tensor_scalar(
                    out=c12[:, 0:1], in0=a_sb, scalar1=-2.0 / N,
                    scalar2=1.0 / N, op0=OP.mult, op1=OP.add)
                nc.vector.tensor_scalar(
                    out=c12[:, 1:2], in0=a_sb, scalar1=1.0 / N,
                    scalar2=None, op0=OP.mult)
                c12_ps = pst([P, 2], "c12ps")
                nc.tensor.matmul(c12_ps, ones_row, c12, start=True, stop=True)
                c12b = consts.tile([P, 2], FP32)
                nc.vector.tensor_copy(c12b, c12_ps)
                return c12b[:, 0:1], c12b[:, 1:2]

            def emit_w16(g, c1_col, c2_col):
                st = ST[g]
                Wt = []
                for b in range(NB):
                    w16 = fpool.tile([P, MC], FP16, tag="w16",
                                     name=f"w16_{g}_{b}")
                    nc.vector.tensor_scalar(
                        out=w16[:, :NF], in0=st["F16"][b], scalar1=c1_col,
                        scalar2=None, op0=OP.mult)
                    nc.vector.tensor_copy(w16[:, NF:MC], c2_col)
                    Wt.append(w16)
                st["W"] = Wt

            def extract_batch(g, kb):
                st = ST[g]
                k0 = kb * KB
                nk = min(KB, K - k0)
                for b in range(NB):
                    w4 = _bc4(st["W"][b][:, :], nk)
                    prod = spool.tile([P, KB, MC], FP32, tag="prod")
                    nc.vector.tensor_tensor(
                        out=prod[:, 0:nk, :], in0=w4,
                        in1=st["mb"][b][:, 0:nk, :], op=OP.mult)
                    nc.vector.tensor_reduce(
                        out=st["Feat"][b][:, k0:k0 + nk],
                        in_=prod[:, 0:nk, :],
                        axis=mybir.AxisListType.X, op=OP.add)

            def step(g, k):
                st = ST[g]
                adjT = st["adjT"]
                sl_prev, sl = (k - 1) % KB, k % KB
                if sl == 0:
                    st["mprev"] = st["mb"]
                    st["mb"] = [
                        mpool.tile([P, KB, MC], FP16, tag="m4",
                                   name=f"mb{g}_{b}_{k // KB}")
                        for b in range(NB)]
                tq = [pst([P, 2, MC], f"tq{g}_{k}_0"),
                      pst([P, 2, MC], f"tq{g}_{k}_1")]
                for ib in range(NB):
                    for jc in range(NB):
                        nc.tensor.matmul(
                            tq[ib // 2][:, ib % 2, :],
                            adjT[jc][:, ib * P:(ib + 1) * P],
                            st["mprev"][jc][:, sl_prev, :],
                            start=(jc == 0), stop=(jc == NB - 1))
                if k == 1:
                    recip = fpool.tile([P, NB], FP32, tag="recip",
                                       name=f"recip{g}")
                    for ib in range(NB):
                        dg = spool.tile([P, 1], FP32, tag="dg")
                        nc.vector.tensor_scalar(
                            out=dg, in0=tq[ib // 2][:, ib % 2, NF:MC],
                            scalar1=1.0, scalar2=None, op0=OP.max)
                        nc.vector.reciprocal(recip[:, ib:ib + 1], dg)
                    st["recip"] = recip
                nact = 2 if g == 0 else 1
                for ib in range(NB):
                    if ib < nact:
                        nc.scalar.activation(
                            out=st["mb"][ib][:, sl, :],
                            in_=tq[ib // 2][:, ib % 2, :], func=AF.Copy,
                            scale=st["recip"][:, ib:ib + 1])
                    else:
                        nc.vector.tensor_scalar(
                            out=st["mb"][ib][:, sl, :],
                            in0=tq[ib // 2][:, ib % 2, :],
                            scalar1=st["recip"][:, ib:ib + 1],
                            scalar2=None, op0=OP.mult)
                if sl == KB - 1:
                    extract_batch(g, k // KB)

            def emit_xt(xb):
                xt = spool.tile([P, DIN], FP32, tag="xt")
                nc.sync.dma_start(out=xt, in_=x_d[xb * P:(xb + 1) * P, :])
                xtp = pst([DIN, P], f"xtp{xb}")
                nc.tensor.transpose(xtp, xt, ident)
                nc.vector.tensor_copy(xT_sb[0:DIN, xb * P:(xb + 1) * P], xtp)

            def emit_hx(xb):
                hxp = pst([P, DX], f"hxp{xb}")
                nc.tensor.matmul(
                    hxp, xT_sb[:, xb * P:(xb + 1) * P], wxb_sb,
                    start=True, stop=True)
                nc.vector.tensor_copy(ots[xb][:, 0:DX], hxp)

            def emit_mlp(g):
                st = ST[g]
                featT = fpool.tile([K, N], FP16, tag="featT", name=f"fT{g}")
                for b in range(NB):
                    ftp = pst([K, P], f"ftp{g}_{b}")
                    nc.tensor.transpose(ftp, st["Feat"][b], ident)
                    nc.scalar.copy(featT[:, b * P:(b + 1) * P], ftp)
                h1p = pst([HID, N], f"h1p{g}")
                nc.tensor.matmul(h1p, w1_16, featT, start=True, stop=True)
                h1 = fpool.tile([HID, N], FP16, tag="h1", name=f"h1{g}")
                nc.scalar.activation(out=h1, in_=h1p, func=AF.Relu, bias=b1_sb)
                h2p = pst([HID, N], f"h2p{g}")
                nc.tensor.matmul(h2p, w2_16, h1, start=True, stop=True)
                h2 = fpool.tile([HID, N], FP16, tag="h2", name=f"h2{g}")
                nc.scalar.activation(out=h2, in_=h2p, func=AF.Relu, bias=b2_sb)
                h3p = pst([DPE, N], f"h3p{g}")
                nc.tensor.matmul(h3p, w3_16, h2, start=True, stop=True)
                h3 = fpool.tile([DPE, N], FP32, tag="h3", name=f"h3{g}")
                nc.scalar.activation(out=h3, in_=h3p, func=AF.Relu, bias=b3_sb)
                for b in range(NB):
                    hp = pst([P, DPE], f"hp{g}_{b}")
                    nc.tensor.transpose(
                        hp, h3[:, b * P:(b + 1) * P], ident[0:DPE, 0:DPE])
                    stats = spool.tile([P, 6], FP32, tag="stats")
                    nc.vector.bn_stats(out=stats, in_=hp)
                    mv = spool.tile([P, 2], FP32, tag="mv")
                    nc.vector.bn_aggr(out=mv, in_=stats)
                    sd = spool.tile([P, 1], FP32, tag="sd")
                    nc.scalar.activation(
                        out=sd, in_=mv[:, 1:2], func=AF.Sqrt, bias=eps_sb)
                    rstd = spool.tile([P, 1], FP32, tag="rstd")
                    nc.vector.reciprocal(rstd, sd)
                    ot = ots[g * NB + b]
                    t0 = spool.tile([P, DPE], FP32, tag="t0")
                    nc.vector.tensor_scalar(
                        out=t0, in0=hp, scalar1=mv[:, 0:1], scalar2=rstd,
                        op0=OP.subtract, op1=OP.mult)
                    t1 = spool.tile([P, DPE], FP32, tag="t1")
                    nc.vector.tensor_tensor(
                        out=t1, in0=t0, in1=ga_sb, op=OP.mult)
                    nc.vector.tensor_tensor(
                        out=ot[:, DX:DOUT], in0=t1, in1=be_sb, op=OP.add)

            # ---------------- schedule ----------------
            emit_prep(0)
            emit_prep(1)
            emit_adj_start(0)
            for c in range(EC):
                emit_adj_chunk(0, c)
            emit_adj_copy(0)
            c1_col, c2_col = emit_c12()
            emit_w16(0, c1_col, c2_col)
            # g1 adjacency interleaved with g0 power-iteration steps
            emit_adj_start(1)
            k0 = 1
            for c in range(EC):
                emit_adj_chunk(1, c)
                if c % 2 == 1 and k0 < K:
                    step(0, k0)
                    k0 += 1
            emit_adj_copy(1)
            emit_w16(1, c1_col, c2_col)
            while k0 < K:
                step(0, k0)
                k0 += 1
            emit_mlp(0)
            # g1 steps interleaved with x-transpose / hx fillers
            def emit_out(xb):
                nc.sync.dma_start(
                    out=out_d[xb * P:(xb + 1) * P, :], in_=ots[xb])

            fillers = [(lambda xb=xb: emit_xt(xb)) for xb in range(XB)]
            for xb in range(XB):
                fillers.append(lambda xb=xb: emit_hx(xb))
                if xb < NB:  # g0 rows: pos_enc already written by emit_mlp(0)
                    fillers.append(lambda xb=xb: emit_out(xb))
            fi = 0
            for k in range(1, K):
                step(1, k)
                if fi < len(fillers):
                    fillers[fi]()
                    fi += 1
            while fi < len(fillers):
                fillers[fi]()
                fi += 1
            emit_mlp(1)
            for xb in range(NB, XB):
                emit_out(xb)

    nc.finalize()
    return nc


_CACHE = {}


def _get_nc():
    if "nc" not in _CACHE:
        _CACHE["nc"] = _build()
    return _CACHE["nc"]


def _shard_inputs(inputs):
    x = np.ascontiguousarray(np.asarray(inputs["x"], dtype=np.float32))
    e = np.ascontiguousarray(np.asarray(inputs["edge_index"], dtype=np.int32))
    fr = np.ascontiguousarray(np.asarray(inputs["fragment_ids"], dtype=np.int32))
    al = np.asarray(inputs["alpha"], dtype=np.float32).reshape(1, 1)
    com = {
        "alpha": al,
        "Wx": np.ascontiguousarray(np.asarray(inputs["Wx"], np.float32)),
        "bx": np.asarray(inputs["bx"], np.float32).reshape(1, DX),
        "W1": np.ascontiguousarray(np.asarray(inputs["W1"], np.float32)),
        "b1": np.asarray(inputs["b1"], np.float32).reshape(HID, 1),
        "W2": np.ascontiguousarray(np.asarray(inputs["W2"], np.float32)),
        "b2": np.asarray(inputs["b2"], np.float32).reshape(HID, 1),
        "W3": np.ascontiguousarray(np.asarray(inputs["W3"], np.float32)),
        "b3": np.asarray(inputs["b3"], np.float32).reshape(DPE, 1),
        "gamma": np.asarray(inputs["gamma"], np.float32).reshape(1, DPE),
        "beta": np.asarray(inputs["beta"], np.float32).reshape(1, DPE),
    }
    in_maps = []
    for c in range(NCORES):
        g0 = c * GPC
        in_maps.append(dict(
            com,
            x=x[g0 * N:(g0 + GPC) * N],
            edges=e[g0:g0 + GPC],
            frags=fr[g0:g0 + GPC],
        ))
    return in_maps


def _run(inputs, trace=False):
    nc = _get_nc()
    in_maps = _shard_inputs(inputs)
    res = run_bass_kernel_spmd(nc, in_maps, list(range(NCORES)), trace=trace)
    out = np.concatenate([res.results[c]["out"] for c in range(NCORES)], axis=0)
    return out, res


def kernel(**inputs):
    out, _ = _run(inputs, trace=False)
    return out
